# revision 8
# baseline (speedup 1.0000x reference)
"""Multi-head self-attention (B=2, S=2048, D=1024, H=16) on 8 TRN2 NeuronCores.

Sharding: core c handles batch b = c//4 and head group g = c%4 (4 heads each).
Each core computes qkv projection for its heads, masked-softmax attention, and
a partial output projection; the host sums the 4 partial outputs per batch.

Scores are computed transposed (keys on partitions, queries on the free dim) so
the P^T tile the PV matmul needs comes straight out of exp() with no transpose.
Softmax skips max-subtraction (scores are O(1) here); the denominator falls out
of a ones-column appended to the V stationary.

Mask handling: each 128key x 512query score tile is classified host-side as
skip / full / partial.  For partial tiles the leading fully-masked query
columns are sliced off the matmul moving dim entirely (for the causal mask
this removes all wasted columns), and only the remaining "mixed" region
(127 cols for causal - one shared pattern) is multiplied by a 0/1 mask after
exp.

Schedule notes (v2):
- PE warm-up: dummy matmuls run during the initial DMA wait so the tensor
  engine's DVFS p-state is at full clock when the first real matmul issues
  (trace showed phase 1 running at 1.2 GHz, half speed, for ~9us).
- Phase-1 inputs stream per-kt-pair (4 w chunks on sync, 4 x chunks on
  scalar) so the kt loop never outruns the DMA stream.
- Element-wise work is spread: exp on Act only; mask-mul, v-copies and the
  first normalize copy on Pool; casts and normalize arithmetic on DVE - the
  Act engine (exp paces attention) and DVE both sat near 90us busy before.
- Tail: strip-3 output projection drains its PSUM tiles through all three of
  Act/DVE/Pool round-robin and DMAs each 512-token block as soon as both
  column halves are done.
"""

from contextlib import ExitStack

import numpy as np

import concourse.bass as bass
import concourse.tile as tile
from concourse import bacc, mybir
from concourse.bass_utils import run_bass_kernel_spmd

F32 = mybir.dt.float32
F16 = mybir.dt.float16

B, S, D, H, DH = 2, 2048, 1024, 16, 64
HPC = 4          # heads per core
NCORES = 8
KT = S // 128    # 16 key tiles of 128
QS = S // 512    # 4 query strips of 512
DKT = D // 128   # 8 contraction tiles for the projections


def _to_f16(x):
    return np.ascontiguousarray(x).astype(np.float16)


def _build(spec, uregw, debug=False):
    """spec[qs][ki] is ('s',) skip | ('f',) full | ('p', pref, regw, mid).

    pref = leading fully-masked query columns (sliced off entirely),
    regw = width of the mixed region right after pref (mask-multiplied),
    mid  = index into the unique mask tiles; uregw[mid] is that tile's width.
    """
    nc = bacc.Bacc()

    mw = sum(uregw) if uregw else 1
    # inputs are pre-tiled partition-major; w/x phase-1 chunks are kt pairs
    wqc = [nc.dram_tensor(f"wqc{i}", [128, 2, 512], F16, kind="ExternalInput")
           for i in range(4)]
    xtc = [nc.dram_tensor(f"xtc{i}", [128, 2, 512], F16, kind="ExternalInput")
           for i in range(4)]
    xrd = [nc.dram_tensor(f"xr{i}", [128, DKT, 512], F16, kind="ExternalInput")
           for i in range(1, 4)]
    wv = nc.dram_tensor("wv", [128, DKT, 256], F16, kind="ExternalInput")
    wo = nc.dram_tensor("wo", [128, 2, D], F16, kind="ExternalInput")
    maskp = nc.dram_tensor("maskp", [128, mw], F16, kind="ExternalInput")
    out = nc.dram_tensor("out", [128, 4 * QS, D], F16, kind="ExternalOutput")

    with tile.TileContext(nc) as tc, ExitStack() as top:
        persist = top.enter_context(tc.tile_pool(name="persist", bufs=1))

        # ---- persistent tiles ----
        # qk[ct]: transposed projections [proj-col, token]; ct 0-1 = q heads
        # (0,1),(2,3) scaled by 1/sqrt(dh) host-side; ct 2-3 = k heads.
        qk = [persist.tile([128, S], F16, name=f"qk{ct}", tag=f"qk{ct}") for ct in range(4)]
        # v_ext: one tile per key strip s; within it key-tile kt=s*4+j and
        # head h sit at column offset 260*j + 65*h, [128 tok, 64 dims + ones]
        v_ext = [
            persist.tile([128, 4 * HPC * 65], F16, name=f"vx{s}", tag=f"vx{s}")
            for s in range(QS)
        ]
        # ot: per (tile t = head pair, strip): [128 head-dims, 512 tokens]
        ot = [
            [
                persist.tile([128, 512], F16, name=f"ot{t}_{s}", tag=f"ot{t}_{s}")
                for s in range(QS)
            ]
            for t in range(2)
        ]
        wo_t = persist.tile([128, 2, D], F16, tag="wo")
        mtiles = [
            persist.tile([128, uregw[m]], F16, name=f"mt{m}", tag=f"mt{m}")
            for m in range(len(uregw))
        ]
        osb = [
            persist.tile([128, 4, D], F16, name=f"osb{i}", tag=f"osb{i}")
            for i in range(2)
        ]
        # dummy warm-up source (memset, never written again)
        dum = persist.tile([128, 256], F16, tag="dum")

        # phase-1 input tiles
        xw = top.enter_context(tc.tile_pool(name="xw", bufs=1))
        xt0 = xw.tile([128, DKT, 512], F16, tag="xt0")
        xr = [
            xw.tile([128, DKT, 512], F16, name=f"xr{i}", tag=f"xr{i}")
            for i in range(1, 4)
        ]
        wqk_t = xw.tile([128, DKT, 512], F16, tag="wqk")
        wv_t = xw.tile([128, DKT, 256], F16, tag="wv")

        # ---- input DMAs: big descriptors, priority-ordered, 3 engines ----
        # warm-up source first so the PE can start immediately
        nc.gpsimd.memset(dum[:], 0.0)
        # ones columns of v_ext, generated on-device (a DMA here costs 8192
        # 2-byte descriptors that jam the queues for ~15us)
        for s in range(QS):
            nc.gpsimd.memset(
                v_ext[s][:].rearrange("p (g c) -> p g c", c=65)[:, :, 64:65], 1.0
            )
        # phase-1 stream: kt-pair chunks, w on sync / x on scalar so each kt
        # pair's operands land together; everything else follows behind
        for i in range(4):
            nc.sync.dma_start(wqk_t[:, 2 * i : 2 * i + 2, :], wqc[i][:])
            nc.scalar.dma_start(xt0[:, 2 * i : 2 * i + 2, :], xtc[i][:])
        nc.gpsimd.dma_start(wv_t[:], wv[:])
        nc.gpsimd.dma_start(xr[0][:], xrd[0][:])
        nc.sync.dma_start(xr[1][:], xrd[1][:])
        nc.gpsimd.dma_start(xr[2][:], xrd[2][:])
        nc.scalar.dma_start(wo_t[:], wo[:])
        moff = 0
        for m in range(len(uregw)):
            nc.sync.dma_start(mtiles[m][:], maskp[:, moff : moff + uregw[m]])
            moff += uregw[m]

        def xslice(kt, lo, hi):
            # columns [lo, hi) of the logical xT tile kt
            if hi <= 512:
                return xt0[:, kt, lo:hi]
            c = lo // 512
            return xr[c - 1][:, kt, lo - 512 * c : hi - 512 * c]

        # ---- phase 1 head: q/k for strip 0, kt-outer so matmuls start on
        # the first DMA chunk.  Warm-up dummies precede it so the PE p-state
        # is ramped by the time the first chunk lands (~10.5us). ----
        with ExitStack() as pha:
            psA = pha.enter_context(tc.tile_pool(name="psA", bufs=1, space="PSUM"))
            wp = psA.tile([128, 512], F32, name="warm", tag="warm")
            for i in range(16):
                nc.tensor.matmul(
                    wp[:, 0:256], dum[:, 0:128], dum[:], start=True, stop=True
                )
            for i in range(12):
                nc.tensor.matmul(
                    wp[:, 256:320], dum[:, 0:128], dum[:, 0:64],
                    start=True, stop=True,
                )
            pa = [
                psA.tile([128, 512], F32, name=f"pa{ct}", tag=f"pa{ct}")
                for ct in range(4)
            ]
            for kt in range(DKT):
                for ct in range(4):
                    nc.tensor.matmul(
                        pa[ct][:],
                        wqk_t[:, kt, 128 * ct : 128 * ct + 128],
                        xt0[:, kt, :],
                        start=(kt == 0),
                        stop=(kt == DKT - 1),
                    )
            for ct in range(4):
                nc.vector.tensor_copy(qk[ct][:, 0:512], pa[ct][:])

        # ---- phase 2: attention, with the remaining projection work
        # (v tiles, q/k strips 1-3, per-strip output projections) emitted as
        # PE filler between attention tile-groups.  The exp() stream on the
        # scalar engine paces attention; the filler keeps the PE busy so the
        # two run concurrently instead of serializing. ----
        with ExitStack() as ph2:
            ptp = ph2.enter_context(tc.tile_pool(name="pt", bufs=4))
            nrm = ph2.enter_context(tc.tile_pool(name="nrm", bufs=3))
            ps_st = ph2.enter_context(
                tc.tile_pool(name="ps_st", bufs=2, space="PSUM")
            )
            ps_o = ph2.enter_context(tc.tile_pool(name="ps_o", bufs=2, space="PSUM"))
            fillp = ph2.enter_context(tc.tile_pool(name="fillp", bufs=2, space="PSUM"))

            def emit_v(st, eng=None):
                # v natural: psum[tok, head*64+d] = xT_tile.T @ wv_tile
                ps = fillp.tile([128, 512], F32, tag="fill")
                for kt in range(DKT):
                    nc.tensor.matmul(
                        ps[:, 0:256],
                        xslice(kt, 128 * st, 128 * st + 128),
                        wv_t[:, kt, :],
                        start=(kt == 0),
                        stop=(kt == DKT - 1),
                    )
                j = st % 4
                dst = v_ext[st // 4][:, 260 * j : 260 * j + 260].rearrange(
                    "p (h c) -> p h c", c=65
                )[:, :, 0:64]
                nc.vector.tensor_copy(
                    dst, ps[:, 0:256].rearrange("p (h c) -> p h c", c=64)
                )

            def emit_qk(ss, ct, half, eng=None):
                # half a q/k strip tile (256 token columns) per filler unit
                # so the units slot between attention groups without starving
                # the exp stream
                ps = fillp.tile([128, 512], F32, tag="fill")
                lo = 512 * ss + 256 * half
                for kt in range(DKT):
                    nc.tensor.matmul(
                        ps[:, 0:256],
                        wqk_t[:, kt, 128 * ct : 128 * ct + 128],
                        xslice(kt, lo, lo + 256),
                        start=(kt == 0),
                        stop=(kt == DKT - 1),
                    )
                nc.vector.tensor_copy(qk[ct][:, lo : lo + 256], ps[:, 0:256])

            def emit_oproj(qs, sti, oc, eng=None):
                ob = osb[qs % 2]
                pop = fillp.tile([128, 512], F32, tag="fill")
                for t in range(2):
                    nc.tensor.matmul(
                        pop[:],
                        ot[t][qs][:, 128 * sti : 128 * sti + 128],
                        wo_t[:, t, 512 * oc : 512 * oc + 512],
                        start=(t == 0),
                        stop=(t == 1),
                    )
                dst = ob[:, sti, 512 * oc : 512 * oc + 512]
                if eng == "scalar":
                    nc.scalar.copy(dst, pop[:])
                else:
                    nc.vector.tensor_copy(dst, pop[:])
                if qs < QS - 1:
                    if sti == 3 and oc == 1:
                        nc.sync.dma_start(out[:, 4 * qs : 4 * qs + 4, :], ob[:])
                elif oc == 1:
                    # last strip: ship each 512-token block as it completes
                    nc.sync.dma_start(
                        out[:, 4 * qs + sti : 4 * qs + sti + 1, :],
                        ob[:, sti : sti + 1, :],
                    )

            def mk(f, *a):
                return lambda **kw: f(*a, **kw)

            # v for keys 0-511 must precede attention strip 0
            for st in range(4):
                emit_v(st)

            # filler due within attention strip qs (deps ready by then;
            # results needed only by strip qs+1)
            fills = {
                0: [mk(emit_qk, 1, ct, hf) for ct in range(4) for hf in range(2)]
                + [mk(emit_v, st) for st in range(4, 8)],
                1: [mk(emit_qk, 2, ct, hf) for ct in range(4) for hf in range(2)]
                + [mk(emit_v, st) for st in range(8, 12)]
                + [mk(emit_oproj, 0, sti, oc) for sti in range(4) for oc in range(2)],
                2: [mk(emit_qk, 3, ct, hf) for ct in range(4) for hf in range(2)]
                + [mk(emit_v, st) for st in range(12, 16)]
                + [mk(emit_oproj, 1, sti, oc) for sti in range(4) for oc in range(2)],
                3: [mk(emit_oproj, 2, sti, oc) for sti in range(4) for oc in range(2)],
            }

            for qs in range(QS):
                partials = []   # (ki, pref, regw, mid)
                valids = []
                for ki in range(KT):
                    st = spec[qs][ki]
                    if st[0] == "f":
                        valids.append(ki)
                    elif st[0] == "p":
                        partials.append((ki, st[1], st[2], st[3]))
                # the PV accumulation must start with a full-width tile
                partials.sort(key=lambda t: t[1])
                assert (not partials) or partials[0][1] == 0, (
                    "need a full-width partial tile to open the accumulation"
                )
                # tile groups of <=2, partials first (j0 opens accumulation)
                groups = [
                    [(ki, pref, regw, mid) for ki, pref, regw, mid in partials[i : i + 2]]
                    for i in range(0, len(partials), 2)
                ]
                groups += [
                    [(ki, 0, 0, -1) for ki in valids[i : i + 2]]
                    for i in range(0, len(valids), 2)
                ]
                n_tiles = len(partials) + len(valids)
                fq = fills[qs]
                # pace filler across the strip's group slots, reserving a few
                # units to cover the end-of-strip normalize latency
                n_slots = HPC * (len(groups) + 1)
                reserve = 4 if qs == QS - 1 else 0
                stride = max(1, n_slots // max(len(fq) - reserve, 1))
                slot = 0

                for h in range(HPC):
                    qT = qk[h // 2][64 * (h % 2) : 64 * (h % 2) + 64, :]
                    kT = qk[2 + h // 2][64 * (h % 2) : 64 * (h % 2) + 64, :]
                    po = ps_o.tile([65, 512], F32, tag="po")

                    # scores/exp for group g run one step ahead of PV for
                    # group g-1 so the PE never sits waiting on exp
                    pending = None
                    npv = 0
                    for g in range(len(groups) + 1):
                        jobs = None
                        if g < len(groups):
                            grp = groups[g]
                            pst = ps_st.tile([128, 1024], F32, tag="pst")
                            pt = ptp.tile([128, 1024], F16, tag="pt")
                            jobs = []
                            for j, (ki, pref, regw, mid) in enumerate(grp):
                                w = 512 - pref
                                nc.tensor.matmul(
                                    pst[:, 512 * j : 512 * j + w],
                                    kT[:, 128 * ki : 128 * ki + 128],
                                    qT[:, 512 * qs + pref : 512 * qs + 512],
                                    start=True,
                                    stop=True,
                                )
                                jobs.append((pt, 512 * j, w, ki, pref))
                            w0 = 512 - grp[0][1]
                            if len(grp) == 1 or w0 == 512:
                                # contiguous region: single exp
                                wlast = 512 * (len(grp) - 1) + 512 - grp[-1][1]
                                nc.scalar.activation(
                                    pt[:, 0:wlast],
                                    pst[:, 0:wlast],
                                    mybir.ActivationFunctionType.Exp,
                                )
                            else:
                                for j, (ki, pref, regw, mid) in enumerate(grp):
                                    w = 512 - pref
                                    nc.scalar.activation(
                                        pt[:, 512 * j : 512 * j + w],
                                        pst[:, 512 * j : 512 * j + w],
                                        mybir.ActivationFunctionType.Exp,
                                    )
                            for j, (ki, pref, regw, mid) in enumerate(grp):
                                if mid >= 0:
                                    nc.gpsimd.tensor_mul(
                                        pt[:, 512 * j : 512 * j + regw],
                                        pt[:, 512 * j : 512 * j + regw],
                                        mtiles[mid][:],
                                    )
                        if (
                            fq
                            and len(fq) > reserve
                            and slot % stride == stride - 1
                        ):
                            fq.pop(0)()
                        slot += 1
                        if pending is not None:
                            for pt_, off, w, ki, pref in pending:
                                vcol = 260 * (ki % 4) + 65 * h
                                nc.tensor.matmul(
                                    po[:, pref : pref + w],
                                    v_ext[ki // 4][:, vcol : vcol + 65],
                                    pt_[:, off : off + w],
                                    start=(npv == 0),
                                    stop=(npv == n_tiles - 1),
                                )
                                npv += 1
                        pending = jobs

                    if h == HPC - 1:
                        # flush leftover filler BEFORE the normalize chain:
                        # anything emitted after it inherits a wait on the
                        # chain's final vector op and sits out ~3us
                        flush_engs = ["scalar", "vector"]
                        fi = 0
                        while fq:
                            fq.pop(0)(eng=flush_engs[fi % 2])
                            fi += 1
                    # normalize: row 64 of po is the softmax denominator
                    # (copy to SBUF first: the custom-DVE reciprocal misreads
                    # PSUM operands on hardware)
                    rden = nrm.tile([1, 512], F32, tag="rden")
                    nc.vector.tensor_copy(rden[:], po[64:65, :])
                    rrec = nrm.tile([1, 512], F32, tag="rrec")
                    nc.vector.reciprocal_approx_fast(rrec[:], rden[:])
                    rb = nrm.tile([64, 512], F32, tag="rb")
                    nc.gpsimd.partition_broadcast(rb[:], rrec[:])
                    nc.vector.tensor_mul(
                        ot[h // 2][qs][64 * (h % 2) : 64 * (h % 2) + 64, :],
                        po[0:64, :],
                        rb[:],
                    )

                # flush filler still due before the next strip starts
                while fq:
                    fq.pop(0)()

            # strip 3's output projection runs after its last head; casts
            # round-robin over the three element-wise engines so the two
            # PSUM buffers drain fast enough to keep the PE's matmul pairs
            # back-to-back, and each 512-token block DMAs out on completion
            tail_engs = ["scalar", "vector"]
            ti = 0
            for sti in range(4):
                for oc in range(2):
                    emit_oproj(3, sti, oc, eng=tail_engs[ti % 2])
                    ti += 1

    nc.finalize()
    return nc


_cache = {}


def _get_nc(key):
    if key not in _cache:
        spec, uregw = key
        _cache[key] = _build([list(r) for r in spec], list(uregw))
    return _cache[key]


def _tile_km(a):
    """[K*128, w] -> [128, K, w] partition-major contiguous."""
    k1, w = a.shape
    return np.ascontiguousarray(
        a.reshape(k1 // 128, 128, w).transpose(1, 0, 2)
    )


def _prepare(x, mask, w_qkv, w_out):
    """Host-side sharding. Returns (cache_key, in_maps)."""
    scale = 1.0 / np.sqrt(DH)

    # classify score tiles from the actual mask, merged across batches so one
    # SPMD program works for all cores.  keep[k, q] = 1 iff key k visible to
    # query q.  A tile is skip if fully masked in every batch, full if fully
    # valid in every batch, else partial with a sliced prefix + mixed region.
    keeps = [(mask[b] != 0).T.astype(np.float32) for b in range(B)]  # [k, q]
    keep_any = np.maximum.reduce(keeps)   # visible in some batch
    keep_all = np.minimum.reduce(keeps)   # visible in every batch

    uniq = {}
    uregw = []
    umask = []
    spec = []
    for qs in range(QS):
        row = []
        for ki in range(KT):
            blk_any = keep_any[128 * ki : 128 * ki + 128, 512 * qs : 512 * qs + 512]
            blk_all = keep_all[128 * ki : 128 * ki + 128, 512 * qs : 512 * qs + 512]
            if blk_any.max() == 0.0:
                row.append(("s",))
                continue
            if blk_all.min() == 1.0:
                row.append(("f",))
                continue
            colm = blk_any.max(axis=0)   # col has any visible key
            colv = blk_all.min(axis=0)   # col fully valid
            nz = np.nonzero(colm)[0]
            pref = int(nz[0]) if len(nz) else 512
            mixed = np.nonzero(colv == 0)[0]
            end = int(mixed[-1]) + 1 if len(mixed) else pref
            regw = max(end - pref, 1)
            regs = tuple(
                k[128 * ki : 128 * ki + 128, 512 * qs + pref : 512 * qs + pref + regw]
                .astype(np.float16)
                .tobytes()
                for k in keeps
            )
            if regs not in uniq:
                uniq[regs] = len(uregw)
                uregw.append(regw)
                umask.append(
                    [
                        np.frombuffer(r, np.float16).reshape(128, regw)
                        for r in regs
                    ]
                )
            row.append(("p", pref, regw, uniq[regs]))
        spec.append(tuple(row))
    key = (tuple(spec), tuple(uregw))

    in_maps = []
    for c in range(NCORES):
        b, g = c // 4, c % 4
        heads = range(4 * g, 4 * g + 4)
        xT = _tile_km(_to_f16(x[b].T))            # [128, 8, 2048]
        wq = np.concatenate(
            [w_qkv[:, 64 * h : 64 * h + 64] for h in heads], axis=1
        ) * scale
        wk = np.concatenate(
            [w_qkv[:, D + 64 * h : D + 64 * h + 64] for h in heads], axis=1
        )
        wvv = np.concatenate(
            [w_qkv[:, 2 * D + 64 * h : 2 * D + 64 * h + 64] for h in heads], axis=1
        )
        woo = np.concatenate(
            [w_out[64 * h : 64 * h + 64, :] for h in heads], axis=0
        )
        wqk = _tile_km(_to_f16(np.concatenate([wq, wk], axis=1)))  # [128, 8, 512]
        if umask:
            mk = np.concatenate([r[b] for r in umask], axis=1).astype(np.float16)
        else:
            mk = np.zeros((128, 1), np.float16)
        im = {
            "wv": _tile_km(_to_f16(wvv)),
            "wo": _tile_km(_to_f16(np.ascontiguousarray(woo))),
            "maskp": np.ascontiguousarray(mk),
        }
        for i in range(4):
            im[f"wqc{i}"] = np.ascontiguousarray(wqk[:, 2 * i : 2 * i + 2, :])
            im[f"xtc{i}"] = np.ascontiguousarray(xT[:, 2 * i : 2 * i + 2, 0:512])
        for i in range(1, 4):
            im[f"xr{i}"] = np.ascontiguousarray(xT[:, :, 512 * i : 512 * i + 512])
        in_maps.append(im)
    return key, in_maps


def _unshuffle_out(o):
    """[128, 16, D] tile-major kernel output -> [S, D]."""
    return np.ascontiguousarray(o.transpose(1, 0, 2)).reshape(S, D)


def _run(x, mask, w_qkv, w_out, trace=False, trace_cores=None):
    key, in_maps = _prepare(x, mask, w_qkv, w_out)
    nc = _get_nc(key)
    res = run_bass_kernel_spmd(
        nc,
        in_maps,
        core_ids=list(range(NCORES)),
        trace=trace,
        trace_cores=trace_cores,
    )
    outs = np.stack(
        [
            sum(
                _unshuffle_out(res.results[4 * b + g]["out"].astype(np.float32))
                for g in range(4)
            )
            for b in range(B)
        ]
    )
    return outs.astype(np.float32), res


def kernel(x, mask, w_qkv, w_out):
    x = np.asarray(x, np.float32)
    mask = np.asarray(mask)
    w_qkv = np.asarray(w_qkv, np.float32)
    w_out = np.asarray(w_out, np.float32)
    out, _ = _run(x, mask, w_qkv, w_out)
    return out


if __name__ == "__main__":
    pass


# revision 10
# speedup vs baseline: 1.9190x; 1.9190x over previous
"""Multi-head self-attention (B=2, S=2048, D=1024, H=16) on 8 TRN2 NeuronCores.

Sharding: core c handles batch b = c//4 and head group g = c%4 (4 heads each).
Each core computes qkv projection for its heads, masked-softmax attention, and
a partial output projection; the host sums the 4 partial outputs per batch.

Scores are computed transposed (keys on partitions, queries on the free dim) so
the P^T tile the PV matmul needs comes straight out of exp() with no transpose.
Softmax skips max-subtraction (scores are O(1) here); the denominator falls out
of a ones-column appended to the V stationary.

Mask handling: each 128key x 512query score tile is classified host-side as
skip / full / partial.  For partial tiles the leading fully-masked query
columns are sliced off the matmul moving dim entirely, and only the remaining
mixed region is multiplied by a 0/1 mask after exp.

Schedule notes (v3):
- PE warm-up: dummy matmuls run during the initial DMA wait so the tensor
  engine's DVFS p-state is at full clock when the first real matmul issues;
  small dummy trickles between phase-1 kt sections keep the clock up while
  the input stream catches up (any idle gap drops the PE to half clock for
  the next ~3us).
- Attention works in supergroups of 4 key tiles: scores for the whole group
  land in one 4-bank PSUM tile (bufs=1), one exp() covers the group
  (tight-packed, so partial tiles cost no extra activate), then 4 PV
  matmuls.  This halves the Act instruction count; Act drops to ~50% duty
  and stops pacing the PE.
- Valid (unmasked) tiles run before partial tiles within each head-strip so
  the first PV of a head never waits on a mask multiply that is queued
  behind the previous head's normalize chain on the DVE queue.
- Tail: strip-3 output projection casts alternate Act/DVE so the two PSUM
  buffers drain fast enough to keep the matmul pairs back-to-back, and each
  512-token block DMAs out as soon as both column halves are done.
"""

from contextlib import ExitStack

import numpy as np

import concourse.bass as bass
import concourse.tile as tile
from concourse import bacc, mybir
from concourse.bass_utils import run_bass_kernel_spmd

F32 = mybir.dt.float32
F16 = mybir.dt.float16

B, S, D, H, DH = 2, 2048, 1024, 16, 64
HPC = 4          # heads per core
NCORES = 8
KT = S // 128    # 16 key tiles of 128
QS = S // 512    # 4 query strips of 512
DKT = D // 128   # 8 contraction tiles for the projections


def _to_f16(x):
    return np.ascontiguousarray(x).astype(np.float16)


def _build(spec, uregw, debug=False):
    """spec[qs][ki] is ('s',) skip | ('f',) full | ('p', pref, regw, mid)."""
    nc = bacc.Bacc()

    mw = sum(uregw) if uregw else 1
    wqc = [nc.dram_tensor(f"wqc{i}", [128, 2, 512], F16, kind="ExternalInput")
           for i in range(4)]
    xtc = [nc.dram_tensor(f"xtc{i}", [128, 2, 512], F16, kind="ExternalInput")
           for i in range(4)]
    xrd = [nc.dram_tensor(f"xr{i}", [128, DKT, 512], F16, kind="ExternalInput")
           for i in range(1, 4)]
    wv = nc.dram_tensor("wv", [128, DKT, 256], F16, kind="ExternalInput")
    wo = nc.dram_tensor("wo", [128, 2, D], F16, kind="ExternalInput")
    maskp = nc.dram_tensor("maskp", [128, mw], F16, kind="ExternalInput")
    out = nc.dram_tensor("out", [128, 4 * QS, D], F16, kind="ExternalOutput")

    with tile.TileContext(nc) as tc, ExitStack() as top:
        persist = top.enter_context(tc.tile_pool(name="persist", bufs=1))

        # ---- persistent tiles ----
        qk = [persist.tile([128, S], F16, name=f"qk{ct}", tag=f"qk{ct}") for ct in range(4)]
        v_ext = [
            persist.tile([128, 4 * HPC * 65], F16, name=f"vx{s}", tag=f"vx{s}")
            for s in range(QS)
        ]
        ot = [
            [
                persist.tile([128, 512], F16, name=f"ot{t}_{s}", tag=f"ot{t}_{s}")
                for s in range(QS)
            ]
            for t in range(2)
        ]
        wo_t = persist.tile([128, 2, D], F16, tag="wo")
        mtiles = [
            persist.tile([128, uregw[m]], F16, name=f"mt{m}", tag=f"mt{m}")
            for m in range(len(uregw))
        ]
        osb = [
            persist.tile([128, 4, D], F16, name=f"osb{i}", tag=f"osb{i}")
            for i in range(2)
        ]
        # dummy warm-up source (memset once, read-only afterwards)
        dum = persist.tile([128, 256], F16, tag="dum")

        # phase-1 input tiles
        xw = top.enter_context(tc.tile_pool(name="xw", bufs=1))
        xt0 = xw.tile([128, DKT, 512], F16, tag="xt0")
        xr = [
            xw.tile([128, DKT, 512], F16, name=f"xr{i}", tag=f"xr{i}")
            for i in range(1, 4)
        ]
        wqk_t = xw.tile([128, DKT, 512], F16, tag="wqk")
        wv_t = xw.tile([128, DKT, 256], F16, tag="wv")

        # ---- input DMAs ----
        nc.gpsimd.memset(dum[:], 0.0)
        # ones columns of v_ext, generated on-device (a DMA here costs 8192
        # 2-byte descriptors that jam the queues for ~15us)
        for s in range(QS):
            nc.gpsimd.memset(
                v_ext[s][:].rearrange("p (g c) -> p g c", c=65)[:, :, 64:65], 1.0
            )
        # phase-1 stream: kt-pair chunks, w/x alternating across the sync and
        # scalar queues so both operands of each kt pair land together and in
        # consumption order
        for i in range(4):
            qw, qx = (nc.sync, nc.scalar) if i % 2 == 0 else (nc.scalar, nc.sync)
            qw.dma_start(wqk_t[:, 2 * i : 2 * i + 2, :], wqc[i][:])
            qx.dma_start(xt0[:, 2 * i : 2 * i + 2, :], xtc[i][:])
        nc.gpsimd.dma_start(wv_t[:], wv[:])
        nc.gpsimd.dma_start(xr[0][:], xrd[0][:])
        nc.sync.dma_start(xr[1][:], xrd[1][:])
        nc.gpsimd.dma_start(xr[2][:], xrd[2][:])
        nc.scalar.dma_start(wo_t[:], wo[:])
        moff = 0
        for m in range(len(uregw)):
            nc.sync.dma_start(mtiles[m][:], maskp[:, moff : moff + uregw[m]])
            moff += uregw[m]

        def xslice(kt, lo, hi):
            if hi <= 512:
                return xt0[:, kt, lo:hi]
            c = lo // 512
            return xr[c - 1][:, kt, lo - 512 * c : hi - 512 * c]

        # ---- phase 1: q/k for strip 0, kt-outer; warm-up dummies ramp the
        # PE clock before the first chunk lands, trickles keep it up when the
        # DMA stream briefly falls behind the kt loop ----
        with ExitStack() as pha:
            psA = pha.enter_context(tc.tile_pool(name="psA", bufs=1, space="PSUM"))
            wp = psA.tile([128, 512], F32, name="warm", tag="warm")

            def dummy(n, cols=256):
                for _ in range(n):
                    nc.tensor.matmul(
                        wp[:, 0:cols], dum[:, 0:128], dum[:, 0:cols],
                        start=True, stop=True,
                    )

            dummy(16)
            dummy(12, cols=64)
            pa = [
                psA.tile([128, 512], F32, name=f"pa{ct}", tag=f"pa{ct}")
                for ct in range(4)
            ]
            for kt in range(DKT):
                for ct in range(4):
                    nc.tensor.matmul(
                        pa[ct][:],
                        wqk_t[:, kt, 128 * ct : 128 * ct + 128],
                        xt0[:, kt, :],
                        start=(kt == 0),
                        stop=(kt == DKT - 1),
                    )
                if kt % 2 == 1 and kt < DKT - 1:
                    # cover the gap until the next kt-pair chunk lands
                    dummy(8, cols=64)
            for ct in range(4):
                nc.vector.tensor_copy(qk[ct][:, 0:512], pa[ct][:])

        # ---- phase 2: attention in supergroups of 4 key tiles, remaining
        # projection work interleaved as PE filler ----
        with ExitStack() as ph2:
            ptp = ph2.enter_context(tc.tile_pool(name="pt", bufs=3))
            nrm = ph2.enter_context(tc.tile_pool(name="nrm", bufs=3))
            ps_st = ph2.enter_context(
                tc.tile_pool(name="ps_st", bufs=1, space="PSUM")
            )
            ps_o = ph2.enter_context(tc.tile_pool(name="ps_o", bufs=2, space="PSUM"))
            fillp = ph2.enter_context(tc.tile_pool(name="fillp", bufs=2, space="PSUM"))

            def emit_v(st, eng=None):
                ps = fillp.tile([128, 512], F32, tag="fill")
                for kt in range(DKT):
                    nc.tensor.matmul(
                        ps[:, 0:256],
                        xslice(kt, 128 * st, 128 * st + 128),
                        wv_t[:, kt, :],
                        start=(kt == 0),
                        stop=(kt == DKT - 1),
                    )
                j = st % 4
                dst = v_ext[st // 4][:, 260 * j : 260 * j + 260].rearrange(
                    "p (h c) -> p h c", c=65
                )[:, :, 0:64]
                nc.vector.tensor_copy(
                    dst, ps[:, 0:256].rearrange("p (h c) -> p h c", c=64)
                )

            def emit_qk(ss, ct, half, eng=None):
                ps = fillp.tile([128, 512], F32, tag="fill")
                lo = 512 * ss + 256 * half
                for kt in range(DKT):
                    nc.tensor.matmul(
                        ps[:, 0:256],
                        wqk_t[:, kt, 128 * ct : 128 * ct + 128],
                        xslice(kt, lo, lo + 256),
                        start=(kt == 0),
                        stop=(kt == DKT - 1),
                    )
                nc.vector.tensor_copy(qk[ct][:, lo : lo + 256], ps[:, 0:256])

            def emit_oproj(qs, sti, oc, eng=None):
                ob = osb[qs % 2]
                pop = fillp.tile([128, 512], F32, tag="fill")
                for t in range(2):
                    nc.tensor.matmul(
                        pop[:],
                        ot[t][qs][:, 128 * sti : 128 * sti + 128],
                        wo_t[:, t, 512 * oc : 512 * oc + 512],
                        start=(t == 0),
                        stop=(t == 1),
                    )
                dst = ob[:, sti, 512 * oc : 512 * oc + 512]
                if eng == "scalar":
                    nc.scalar.copy(dst, pop[:])
                else:
                    nc.vector.tensor_copy(dst, pop[:])
                if qs < QS - 1:
                    if sti == 3 and oc == 1:
                        nc.sync.dma_start(out[:, 4 * qs : 4 * qs + 4, :], ob[:])
                elif oc == 1:
                    nc.sync.dma_start(
                        out[:, 4 * qs + sti : 4 * qs + sti + 1, :],
                        ob[:, sti : sti + 1, :],
                    )

            def mk(f, *a):
                return lambda **kw: f(*a, **kw)

            # v for keys 0-511 must precede attention strip 0
            for st in range(4):
                emit_v(st)

            fills = {
                0: [mk(emit_qk, 1, ct, hf) for ct in range(4) for hf in range(2)]
                + [mk(emit_v, st) for st in range(4, 8)],
                1: [mk(emit_qk, 2, ct, hf) for ct in range(4) for hf in range(2)]
                + [mk(emit_v, st) for st in range(8, 12)]
                + [mk(emit_oproj, 0, sti, oc) for sti in range(4) for oc in range(2)],
                2: [mk(emit_qk, 3, ct, hf) for ct in range(4) for hf in range(2)]
                + [mk(emit_v, st) for st in range(12, 16)]
                + [mk(emit_oproj, 1, sti, oc) for sti in range(4) for oc in range(2)],
                3: [mk(emit_oproj, 2, sti, oc) for sti in range(4) for oc in range(2)],
            }

            for qs in range(QS):
                partials = []   # (ki, pref, regw, mid)
                valids = []
                for ki in range(KT):
                    st = spec[qs][ki]
                    if st[0] == "f":
                        valids.append(ki)
                    elif st[0] == "p":
                        partials.append((ki, st[1], st[2], st[3]))
                partials.sort(key=lambda t: t[1])
                # valid tiles first: the opening PV write must span the full
                # strip (pref 0), and partial tiles' mask multiplies then sit
                # well clear of the PV matmuls that consume them
                tiles = [(ki, 0, 0, -1) for ki in valids] + partials
                assert tiles and tiles[0][1] == 0, "need a full-width opener"
                # supergroups of <=4 tiles, <=2048 packed score columns
                sgs = []
                cur, curw = [], 0
                for t in tiles:
                    w = 512 - t[1]
                    if len(cur) == 4 or curw + w > 2048:
                        sgs.append(cur)
                        cur, curw = [], 0
                    cur.append(t)
                    curw += w
                if cur:
                    sgs.append(cur)
                n_tiles = len(tiles)
                fq = fills[qs]
                n_slots = HPC * (len(sgs) + 1)
                reserve = 4 if qs == QS - 1 else 0
                slots_left = n_slots

                for h in range(HPC):
                    qT = qk[h // 2][64 * (h % 2) : 64 * (h % 2) + 64, :]
                    kT = qk[2 + h // 2][64 * (h % 2) : 64 * (h % 2) + 64, :]
                    po = ps_o.tile([65, 512], F32, tag="po")

                    pending = None
                    npv = 0
                    for g in range(len(sgs) + 1):
                        jobs = None
                        if g < len(sgs):
                            sg = sgs[g]
                            pst = ps_st.tile([128, 2048], F32, tag="pst")
                            pt = ptp.tile([128, 2048], F16, tag="pt")
                            jobs = []
                            off = 0
                            for ki, pref, regw, mid in sg:
                                w = 512 - pref
                                if off // 512 != (off + w - 1) // 512:
                                    # a single matmul's PSUM write must stay
                                    # inside one 2KB bank
                                    off = (off + 511) // 512 * 512
                                nc.tensor.matmul(
                                    pst[:, off : off + w],
                                    kT[:, 128 * ki : 128 * ki + 128],
                                    qT[:, 512 * qs + pref : 512 * qs + 512],
                                    start=True,
                                    stop=True,
                                )
                                jobs.append((pt, off, w, ki, pref, regw, mid))
                                off += w
                            nc.scalar.activation(
                                pt[:, 0:off],
                                pst[:, 0:off],
                                mybir.ActivationFunctionType.Exp,
                            )
                            for pt_, o, w, ki, pref, regw, mid in jobs:
                                if mid >= 0:
                                    nc.vector.tensor_mul(
                                        pt_[:, o : o + regw],
                                        pt_[:, o : o + regw],
                                        mtiles[mid][:],
                                    )
                        # adaptive filler drain: spread what's left over the
                        # remaining slots of this strip
                        avail = len(fq) - reserve
                        if avail > 0:
                            want = -(-avail // slots_left)   # ceil
                            for _ in range(min(want, avail)):
                                fq.pop(0)()
                        slots_left -= 1
                        if pending is not None:
                            for pt_, o, w, ki, pref, regw, mid in pending:
                                vcol = 260 * (ki % 4) + 65 * h
                                nc.tensor.matmul(
                                    po[:, pref : pref + w],
                                    v_ext[ki // 4][:, vcol : vcol + 65],
                                    pt_[:, o : o + w],
                                    start=(npv == 0),
                                    stop=(npv == n_tiles - 1),
                                )
                                npv += 1
                        pending = jobs

                    if h == HPC - 1:
                        # flush leftover filler BEFORE the normalize chain
                        flush_engs = ["scalar", "vector"]
                        fi = 0
                        while fq:
                            fq.pop(0)(eng=flush_engs[fi % 2])
                            fi += 1
                    # normalize: row 64 of po is the softmax denominator
                    # (copy to SBUF first: the custom-DVE reciprocal misreads
                    # PSUM operands on hardware)
                    rden = nrm.tile([1, 512], F32, tag="rden")
                    nc.vector.tensor_copy(rden[:], po[64:65, :])
                    rrec = nrm.tile([1, 512], F32, tag="rrec")
                    nc.vector.reciprocal_approx_fast(rrec[:], rden[:])
                    rb = nrm.tile([64, 512], F32, tag="rb")
                    nc.gpsimd.partition_broadcast(rb[:], rrec[:])
                    nc.vector.tensor_mul(
                        ot[h // 2][qs][64 * (h % 2) : 64 * (h % 2) + 64, :],
                        po[0:64, :],
                        rb[:],
                    )

                while fq:
                    fq.pop(0)()

            # strip-3 output projection: casts alternate Act/DVE, per-block
            # output DMA
            tail_engs = ["scalar", "vector"]
            ti = 0
            for sti in range(4):
                for oc in range(2):
                    emit_oproj(3, sti, oc, eng=tail_engs[ti % 2])
                    ti += 1

    nc.finalize()
    return nc


_cache = {}


def _get_nc(key):
    if key not in _cache:
        spec, uregw = key
        _cache[key] = _build([list(r) for r in spec], list(uregw))
    return _cache[key]


def _tile_km(a):
    """[K*128, w] -> [128, K, w] partition-major contiguous."""
    k1, w = a.shape
    return np.ascontiguousarray(
        a.reshape(k1 // 128, 128, w).transpose(1, 0, 2)
    )


def _prepare(x, mask, w_qkv, w_out):
    """Host-side sharding. Returns (cache_key, in_maps)."""
    scale = 1.0 / np.sqrt(DH)

    keeps = [(mask[b] != 0).T.astype(np.float32) for b in range(B)]  # [k, q]
    keep_any = np.maximum.reduce(keeps)
    keep_all = np.minimum.reduce(keeps)

    uniq = {}
    uregw = []
    umask = []
    spec = []
    for qs in range(QS):
        row = []
        for ki in range(KT):
            blk_any = keep_any[128 * ki : 128 * ki + 128, 512 * qs : 512 * qs + 512]
            blk_all = keep_all[128 * ki : 128 * ki + 128, 512 * qs : 512 * qs + 512]
            if blk_any.max() == 0.0:
                row.append(("s",))
                continue
            if blk_all.min() == 1.0:
                row.append(("f",))
                continue
            colm = blk_any.max(axis=0)
            colv = blk_all.min(axis=0)
            nz = np.nonzero(colm)[0]
            pref = int(nz[0]) if len(nz) else 512
            mixed = np.nonzero(colv == 0)[0]
            end = int(mixed[-1]) + 1 if len(mixed) else pref
            regw = max(end - pref, 1)
            regs = tuple(
                k[128 * ki : 128 * ki + 128, 512 * qs + pref : 512 * qs + pref + regw]
                .astype(np.float16)
                .tobytes()
                for k in keeps
            )
            if regs not in uniq:
                uniq[regs] = len(uregw)
                uregw.append(regw)
                umask.append(
                    [
                        np.frombuffer(r, np.float16).reshape(128, regw)
                        for r in regs
                    ]
                )
            row.append(("p", pref, regw, uniq[regs]))
        spec.append(tuple(row))
    key = (tuple(spec), tuple(uregw))

    in_maps = []
    for c in range(NCORES):
        b, g = c // 4, c % 4
        heads = range(4 * g, 4 * g + 4)
        xT = _tile_km(_to_f16(x[b].T))            # [128, 8, 2048]
        wq = np.concatenate(
            [w_qkv[:, 64 * h : 64 * h + 64] for h in heads], axis=1
        ) * scale
        wk = np.concatenate(
            [w_qkv[:, D + 64 * h : D + 64 * h + 64] for h in heads], axis=1
        )
        wvv = np.concatenate(
            [w_qkv[:, 2 * D + 64 * h : 2 * D + 64 * h + 64] for h in heads], axis=1
        )
        woo = np.concatenate(
            [w_out[64 * h : 64 * h + 64, :] for h in heads], axis=0
        )
        wqk = _tile_km(_to_f16(np.concatenate([wq, wk], axis=1)))  # [128, 8, 512]
        if umask:
            mk = np.concatenate([r[b] for r in umask], axis=1).astype(np.float16)
        else:
            mk = np.zeros((128, 1), np.float16)
        im = {
            "wv": _tile_km(_to_f16(wvv)),
            "wo": _tile_km(_to_f16(np.ascontiguousarray(woo))),
            "maskp": np.ascontiguousarray(mk),
        }
        for i in range(4):
            im[f"wqc{i}"] = np.ascontiguousarray(wqk[:, 2 * i : 2 * i + 2, :])
            im[f"xtc{i}"] = np.ascontiguousarray(xT[:, 2 * i : 2 * i + 2, 0:512])
        for i in range(1, 4):
            im[f"xr{i}"] = np.ascontiguousarray(xT[:, :, 512 * i : 512 * i + 512])
        in_maps.append(im)
    return key, in_maps


def _unshuffle_out(o):
    """[128, 16, D] tile-major kernel output -> [S, D]."""
    return np.ascontiguousarray(o.transpose(1, 0, 2)).reshape(S, D)


def _run(x, mask, w_qkv, w_out, trace=False, trace_cores=None):
    key, in_maps = _prepare(x, mask, w_qkv, w_out)
    nc = _get_nc(key)
    res = run_bass_kernel_spmd(
        nc,
        in_maps,
        core_ids=list(range(NCORES)),
        trace=trace,
        trace_cores=trace_cores,
    )
    outs = np.stack(
        [
            sum(
                _unshuffle_out(res.results[4 * b + g]["out"].astype(np.float32))
                for g in range(4)
            )
            for b in range(B)
        ]
    )
    return outs.astype(np.float32), res


def kernel(x, mask, w_qkv, w_out):
    x = np.asarray(x, np.float32)
    mask = np.asarray(mask)
    w_qkv = np.asarray(w_qkv, np.float32)
    w_out = np.asarray(w_out, np.float32)
    out, _ = _run(x, mask, w_qkv, w_out)
    return out


# revision 16
# speedup vs baseline: 2.2109x; 1.1521x over previous
"""Multi-head self-attention (B=2, S=2048, D=1024, H=16) on 8 TRN2 NeuronCores.

Sharding: core c handles batch b = c//4 and head group g = c%4 (4 heads each).
Each core computes qkv projection for its heads, masked-softmax attention, and
a partial output projection; the host sums the 4 partial outputs per batch.

Scores are computed transposed (keys on partitions, queries on the free dim) so
the P^T tile the PV matmul needs comes straight out of exp() with no transpose.
Softmax skips max-subtraction (scores are O(1) here); the denominator falls out
of a ones-column appended to the V stationary.

Mask handling: each 128key x 512query score tile is classified host-side as
skip / full / partial.  For partial tiles the leading fully-masked query
columns are sliced off the matmul moving dim entirely, and only the remaining
mixed region is multiplied by a 0/1 mask after exp.

Schedule notes (v3):
- PE warm-up: dummy matmuls run during the initial DMA wait so the tensor
  engine's DVFS p-state is at full clock when the first real matmul issues;
  small dummy trickles between phase-1 kt sections keep the clock up while
  the input stream catches up (any idle gap drops the PE to half clock for
  the next ~3us).
- Attention works in supergroups of 4 key tiles: scores for the whole group
  land in one 4-bank PSUM tile (bufs=1), one exp() covers the group
  (tight-packed, so partial tiles cost no extra activate), then 4 PV
  matmuls.  This halves the Act instruction count; Act drops to ~50% duty
  and stops pacing the PE.
- Valid (unmasked) tiles run before partial tiles within each head-strip so
  the first PV of a head never waits on a mask multiply that is queued
  behind the previous head's normalize chain on the DVE queue.
- Tail: strip-3 output projection casts alternate Act/DVE so the two PSUM
  buffers drain fast enough to keep the matmul pairs back-to-back, and each
  512-token block DMAs out as soon as both column halves are done.
"""

from contextlib import ExitStack

import numpy as np

import concourse.bass as bass
import concourse.tile as tile
from concourse import bacc, mybir
from concourse.bass_utils import run_bass_kernel_spmd

F32 = mybir.dt.float32
F16 = mybir.dt.float16

B, S, D, H, DH = 2, 2048, 1024, 16, 64
HPC = 4          # heads per core
NCORES = 8
KT = S // 128    # 16 key tiles of 128
QS = S // 512    # 4 query strips of 512
DKT = D // 128   # 8 contraction tiles for the projections


def _to_f16(x):
    return np.ascontiguousarray(x).astype(np.float16)


def _build(spec, uregw, debug=False):
    """spec[qs][ki] is ('s',) skip | ('f',) full | ('p', pref, regw, mid)."""
    nc = bacc.Bacc()

    mw = sum(uregw) if uregw else 1
    wqc = [nc.dram_tensor(f"wqc{i}", [128, 2, 512], F16, kind="ExternalInput")
           for i in range(4)]
    xtc = [nc.dram_tensor(f"xtc{i}", [128, 2, 512], F16, kind="ExternalInput")
           for i in range(4)]
    xrd = [nc.dram_tensor(f"xr{i}", [128, DKT, 512], F16, kind="ExternalInput")
           for i in range(1, 4)]
    wv = nc.dram_tensor("wv", [128, DKT, 256], F16, kind="ExternalInput")
    wo = nc.dram_tensor("wo", [128, 2, D], F16, kind="ExternalInput")
    maskp = nc.dram_tensor("maskp", [128, mw], F16, kind="ExternalInput")
    out = nc.dram_tensor("out", [128, 4 * QS, D], F16, kind="ExternalOutput")

    with tile.TileContext(nc) as tc, ExitStack() as top:
        persist = top.enter_context(tc.tile_pool(name="persist", bufs=1))

        # ---- persistent tiles ----
        qk = [persist.tile([128, S], F16, name=f"qk{ct}", tag=f"qk{ct}") for ct in range(4)]
        v_ext = [
            persist.tile([128, 4 * HPC * 65], F16, name=f"vx{s}", tag=f"vx{s}")
            for s in range(QS)
        ]
        ot = [
            [
                persist.tile([128, 512], F16, name=f"ot{t}_{s}", tag=f"ot{t}_{s}")
                for s in range(QS)
            ]
            for t in range(2)
        ]
        wo_t = persist.tile([128, 2, D], F16, tag="wo")
        mtiles = [
            persist.tile([128, uregw[m]], F16, name=f"mt{m}", tag=f"mt{m}")
            for m in range(len(uregw))
        ]
        osb = [
            persist.tile([128, 4, D], F16, name=f"osb{i}", tag=f"osb{i}")
            for i in range(2)
        ]
        # dummy warm-up source (memset once, read-only afterwards)
        dum = persist.tile([128, 256], F16, tag="dum")

        # phase-1 input tiles
        xw = top.enter_context(tc.tile_pool(name="xw", bufs=1))
        xt0 = xw.tile([128, DKT, 512], F16, tag="xt0")
        xr = [
            xw.tile([128, DKT, 512], F16, name=f"xr{i}", tag=f"xr{i}")
            for i in range(1, 4)
        ]
        wqk_t = xw.tile([128, DKT, 512], F16, tag="wqk")
        wv_t = xw.tile([128, DKT, 256], F16, tag="wv")

        # ---- input DMAs ----
        nc.gpsimd.memset(dum[:], 0.0)
        # ones columns of v_ext, generated on-device (a DMA here costs 8192
        # 2-byte descriptors that jam the queues for ~15us)
        for s in range(QS):
            nc.gpsimd.memset(
                v_ext[s][:].rearrange("p (g c) -> p g c", c=65)[:, :, 64:65], 1.0
            )
        # phase-1 stream rides the software-dynamic (gpsimd) queue - it
        # sustains ~220 GB/s where the per-engine hardware queues top out
        # near 80 - as interleaved w/x kt-pair chunks in consumption order
        for i in range(4):
            nc.gpsimd.dma_start(wqk_t[:, 2 * i : 2 * i + 2, :], wqc[i][:])
            nc.gpsimd.dma_start(xt0[:, 2 * i : 2 * i + 2, :], xtc[i][:])
        nc.gpsimd.dma_start(xr[0][:], xrd[0][:])
        nc.scalar.dma_start(wv_t[:], wv[:])
        nc.scalar.dma_start(wo_t[:], wo[:])
        nc.sync.dma_start(xr[1][:], xrd[1][:])
        nc.scalar.dma_start(xr[2][:], xrd[2][:])
        moff = 0
        for m in range(len(uregw)):
            nc.sync.dma_start(mtiles[m][:], maskp[:, moff : moff + uregw[m]])
            moff += uregw[m]

        def xslice(kt, lo, hi):
            if hi <= 512:
                return xt0[:, kt, lo:hi]
            c = lo // 512
            return xr[c - 1][:, kt, lo - 512 * c : hi - 512 * c]

        # ---- phase 1: q/k for strip 0, kt-outer; warm-up dummies ramp the
        # PE clock before the first chunk lands, trickles keep it up when the
        # DMA stream briefly falls behind the kt loop ----
        with ExitStack() as pha:
            psA = pha.enter_context(tc.tile_pool(name="psA", bufs=1, space="PSUM"))
            wp = psA.tile([128, 512], F32, name="warm", tag="warm")

            def dummy(n, cols=256):
                for _ in range(n):
                    nc.tensor.matmul(
                        wp[:, 0:cols], dum[:, 0:128], dum[:, 0:cols],
                        start=True, stop=True,
                    )

            dummy(16)
            dummy(12, cols=64)
            pa = [
                psA.tile([128, 512], F32, name=f"pa{ct}", tag=f"pa{ct}")
                for ct in range(4)
            ]
            for kt in range(DKT):
                for ct in range(4):
                    nc.tensor.matmul(
                        pa[ct][:],
                        wqk_t[:, kt, 128 * ct : 128 * ct + 128],
                        xt0[:, kt, :],
                        start=(kt == 0),
                        stop=(kt == DKT - 1),
                    )
                if kt % 2 == 1 and kt < DKT - 1:
                    # cover the gap until the next kt-pair chunk lands
                    dummy(8, cols=64)
            for ct in range(4):
                nc.vector.tensor_copy(qk[ct][:, 0:512], pa[ct][:])

        # ---- phase 2: attention in supergroups of 4 key tiles, remaining
        # projection work interleaved as PE filler ----
        with ExitStack() as ph2:
            ptp = ph2.enter_context(tc.tile_pool(name="pt", bufs=4))
            nrm = ph2.enter_context(tc.tile_pool(name="nrm", bufs=3))
            ps_st = ph2.enter_context(
                tc.tile_pool(name="ps_st", bufs=2, space="PSUM")
            )
            ps_o = ph2.enter_context(tc.tile_pool(name="ps_o", bufs=2, space="PSUM"))
            fillp = ph2.enter_context(tc.tile_pool(name="fillp", bufs=2, space="PSUM"))

            def emit_v(st, eng=None):
                ps = fillp.tile([128, 512], F32, tag="fill")
                for kt in range(DKT):
                    nc.tensor.matmul(
                        ps[:, 0:256],
                        xslice(kt, 128 * st, 128 * st + 128),
                        wv_t[:, kt, :],
                        start=(kt == 0),
                        stop=(kt == DKT - 1),
                    )
                j = st % 4
                dst = v_ext[st // 4][:, 260 * j : 260 * j + 260].rearrange(
                    "p (h c) -> p h c", c=65
                )[:, :, 0:64]
                nc.vector.tensor_copy(
                    dst, ps[:, 0:256].rearrange("p (h c) -> p h c", c=64)
                )

            def emit_qk(ss, ct, half, eng=None):
                ps = fillp.tile([128, 512], F32, tag="fill")
                lo = 512 * ss + 256 * half
                for kt in range(DKT):
                    nc.tensor.matmul(
                        ps[:, 0:256],
                        wqk_t[:, kt, 128 * ct : 128 * ct + 128],
                        xslice(kt, lo, lo + 256),
                        start=(kt == 0),
                        stop=(kt == DKT - 1),
                    )
                nc.vector.tensor_copy(qk[ct][:, lo : lo + 256], ps[:, 0:256])

            def emit_oproj(qs, sti, oc, eng=None):
                ob = osb[qs % 2]
                pop = fillp.tile([128, 512], F32, tag="fill")
                for t in range(2):
                    nc.tensor.matmul(
                        pop[:],
                        ot[t][qs][:, 128 * sti : 128 * sti + 128],
                        wo_t[:, t, 512 * oc : 512 * oc + 512],
                        start=(t == 0),
                        stop=(t == 1),
                    )
                dst = ob[:, sti, 512 * oc : 512 * oc + 512]
                if eng == "scalar":
                    nc.scalar.copy(dst, pop[:])
                else:
                    nc.vector.tensor_copy(dst, pop[:])
                if qs < QS - 1:
                    if sti == 3 and oc == 1:
                        nc.sync.dma_start(out[:, 4 * qs : 4 * qs + 4, :], ob[:])
                elif oc == 1:
                    nc.sync.dma_start(
                        out[:, 4 * qs + sti : 4 * qs + sti + 1, :],
                        ob[:, sti : sti + 1, :],
                    )

            def mk(f, *a):
                return lambda **kw: f(*a, **kw)

            # v for keys 0-511 must precede attention strip 0
            for st in range(4):
                emit_v(st)

            fills = {
                0: [mk(emit_qk, 1, ct, hf) for ct in range(4) for hf in range(2)]
                + [mk(emit_v, st) for st in range(4, 8)],
                1: [mk(emit_qk, 2, ct, hf) for ct in range(4) for hf in range(2)]
                + [mk(emit_v, st) for st in range(8, 12)]
                + [mk(emit_oproj, 0, sti, oc) for sti in range(4) for oc in range(2)],
                2: [mk(emit_qk, 3, ct, hf) for ct in range(4) for hf in range(2)]
                + [mk(emit_v, st) for st in range(12, 16)]
                + [mk(emit_oproj, 1, sti, oc) for sti in range(4) for oc in range(2)],
                3: [mk(emit_oproj, 2, sti, oc) for sti in range(4) for oc in range(2)],
            }

            for qs in range(QS):
                partials = []   # (ki, pref, regw, mid)
                valids = []
                for ki in range(KT):
                    st = spec[qs][ki]
                    if st[0] == "f":
                        valids.append(ki)
                    elif st[0] == "p":
                        partials.append((ki, st[1], st[2], st[3]))
                partials.sort(key=lambda t: t[1])
                # valid tiles first: the opening PV write must span the full
                # strip (pref 0), and partial tiles' mask multiplies then sit
                # well clear of the PV matmuls that consume them
                tiles = [(ki, 0, 0, -1) for ki in valids] + partials
                assert tiles and tiles[0][1] == 0, "need a full-width opener"
                sgs = [tiles[i : i + 2] for i in range(0, len(tiles), 2)]
                n_tiles = len(tiles)
                fq = fills[qs]
                n_slots = HPC * (len(sgs) + 1)
                reserve = 4 if qs == QS - 1 else 0
                # strip 0's fillers read x tokens 512-1023, which are still
                # in flight on the DMA stream when the strip opens
                defer = 4 if qs == 0 else 0
                slot = 0
                slots_left = n_slots - defer

                for h in range(HPC):
                    qT = qk[h // 2][64 * (h % 2) : 64 * (h % 2) + 64, :]
                    kT = qk[2 + h // 2][64 * (h % 2) : 64 * (h % 2) + 64, :]
                    po = ps_o.tile([65, 512], F32, tag="po")

                    pending = None
                    npv = 0
                    for g in range(len(sgs) + 1):
                        jobs = None
                        if g < len(sgs):
                            sg = sgs[g]
                            pst = ps_st.tile([128, 1024], F32, tag="pst")
                            pt = ptp.tile([128, 1024], F16, tag="pt")
                            jobs = []
                            off = 0
                            for ki, pref, regw, mid in sg:
                                w = 512 - pref
                                if off // 512 != (off + w - 1) // 512:
                                    # a single matmul's PSUM write must stay
                                    # inside one 2KB bank
                                    off = (off + 511) // 512 * 512
                                nc.tensor.matmul(
                                    pst[:, off : off + w],
                                    kT[:, 128 * ki : 128 * ki + 128],
                                    qT[:, 512 * qs + pref : 512 * qs + 512],
                                    start=True,
                                    stop=True,
                                )
                                jobs.append((pt, off, w, ki, pref, regw, mid))
                                off += w
                            # tight packing keeps this a single exp even for
                            # the partial (diagonal) pairs
                            nc.scalar.activation(
                                pt[:, 0:off],
                                pst[:, 0:off],
                                mybir.ActivationFunctionType.Exp,
                            )
                            for pt_, o, w, ki, pref, regw, mid in jobs:
                                if mid >= 0:
                                    nc.vector.tensor_mul(
                                        pt_[:, o : o + regw],
                                        pt_[:, o : o + regw],
                                        mtiles[mid][:],
                                    )
                        # adaptive filler drain: spread what's left over the
                        # remaining slots of this strip
                        if slot >= defer:
                            avail = len(fq) - reserve
                            if avail > 0:
                                want = -(-avail // slots_left)   # ceil
                                for _ in range(min(want, avail)):
                                    fq.pop(0)()
                            slots_left -= 1
                        slot += 1
                        if pending is not None:
                            for pt_, o, w, ki, pref, regw, mid in pending:
                                vcol = 260 * (ki % 4) + 65 * h
                                nc.tensor.matmul(
                                    po[:, pref : pref + w],
                                    v_ext[ki // 4][:, vcol : vcol + 65],
                                    pt_[:, o : o + w],
                                    start=(npv == 0),
                                    stop=(npv == n_tiles - 1),
                                )
                                npv += 1
                        pending = jobs

                    if h == HPC - 1:
                        # flush leftover filler BEFORE the normalize chain
                        flush_engs = ["scalar", "vector"]
                        fi = 0
                        while fq:
                            fq.pop(0)(eng=flush_engs[fi % 2])
                            fi += 1
                    # normalize: row 64 of po is the softmax denominator
                    # (copy to SBUF first: the custom-DVE reciprocal misreads
                    # PSUM operands on hardware)
                    rden = nrm.tile([1, 512], F32, tag="rden")
                    nc.vector.tensor_copy(rden[:], po[64:65, :])
                    rrec = nrm.tile([1, 512], F32, tag="rrec")
                    nc.vector.reciprocal_approx_fast(rrec[:], rden[:])
                    rb = nrm.tile([64, 512], F32, tag="rb")
                    nc.gpsimd.partition_broadcast(rb[:], rrec[:])
                    nc.vector.tensor_mul(
                        ot[h // 2][qs][64 * (h % 2) : 64 * (h % 2) + 64, :],
                        po[0:64, :],
                        rb[:],
                    )

                while fq:
                    fq.pop(0)()

            # strip-3 output projection: casts alternate Act/DVE, per-block
            # output DMA
            tail_engs = ["scalar", "vector"]
            ti = 0
            for sti in range(4):
                for oc in range(2):
                    emit_oproj(3, sti, oc, eng=tail_engs[ti % 2])
                    ti += 1

    nc.finalize()
    return nc


_cache = {}


def _get_nc(key):
    if key not in _cache:
        spec, uregw = key
        _cache[key] = _build([list(r) for r in spec], list(uregw))
    return _cache[key]


def _tile_km(a):
    """[K*128, w] -> [128, K, w] partition-major contiguous."""
    k1, w = a.shape
    return np.ascontiguousarray(
        a.reshape(k1 // 128, 128, w).transpose(1, 0, 2)
    )


def _prepare(x, mask, w_qkv, w_out):
    """Host-side sharding. Returns (cache_key, in_maps)."""
    scale = 1.0 / np.sqrt(DH)

    keeps = [(mask[b] != 0).T.astype(np.float32) for b in range(B)]  # [k, q]
    keep_any = np.maximum.reduce(keeps)
    keep_all = np.minimum.reduce(keeps)

    uniq = {}
    uregw = []
    umask = []
    spec = []
    for qs in range(QS):
        row = []
        for ki in range(KT):
            blk_any = keep_any[128 * ki : 128 * ki + 128, 512 * qs : 512 * qs + 512]
            blk_all = keep_all[128 * ki : 128 * ki + 128, 512 * qs : 512 * qs + 512]
            if blk_any.max() == 0.0:
                row.append(("s",))
                continue
            if blk_all.min() == 1.0:
                row.append(("f",))
                continue
            colm = blk_any.max(axis=0)
            colv = blk_all.min(axis=0)
            nz = np.nonzero(colm)[0]
            pref = int(nz[0]) if len(nz) else 512
            mixed = np.nonzero(colv == 0)[0]
            end = int(mixed[-1]) + 1 if len(mixed) else pref
            regw = max(end - pref, 1)
            regs = tuple(
                k[128 * ki : 128 * ki + 128, 512 * qs + pref : 512 * qs + pref + regw]
                .astype(np.float16)
                .tobytes()
                for k in keeps
            )
            if regs not in uniq:
                uniq[regs] = len(uregw)
                uregw.append(regw)
                umask.append(
                    [
                        np.frombuffer(r, np.float16).reshape(128, regw)
                        for r in regs
                    ]
                )
            row.append(("p", pref, regw, uniq[regs]))
        spec.append(tuple(row))
    key = (tuple(spec), tuple(uregw))

    in_maps = []
    for c in range(NCORES):
        b, g = c // 4, c % 4
        heads = range(4 * g, 4 * g + 4)
        xT = _tile_km(_to_f16(x[b].T))            # [128, 8, 2048]
        wq = np.concatenate(
            [w_qkv[:, 64 * h : 64 * h + 64] for h in heads], axis=1
        ) * scale
        wk = np.concatenate(
            [w_qkv[:, D + 64 * h : D + 64 * h + 64] for h in heads], axis=1
        )
        wvv = np.concatenate(
            [w_qkv[:, 2 * D + 64 * h : 2 * D + 64 * h + 64] for h in heads], axis=1
        )
        woo = np.concatenate(
            [w_out[64 * h : 64 * h + 64, :] for h in heads], axis=0
        )
        wqk = _tile_km(_to_f16(np.concatenate([wq, wk], axis=1)))  # [128, 8, 512]
        if umask:
            mk = np.concatenate([r[b] for r in umask], axis=1).astype(np.float16)
        else:
            mk = np.zeros((128, 1), np.float16)
        im = {
            "wv": _tile_km(_to_f16(wvv)),
            "wo": _tile_km(_to_f16(np.ascontiguousarray(woo))),
            "maskp": np.ascontiguousarray(mk),
        }
        for i in range(4):
            im[f"wqc{i}"] = np.ascontiguousarray(wqk[:, 2 * i : 2 * i + 2, :])
            im[f"xtc{i}"] = np.ascontiguousarray(xT[:, 2 * i : 2 * i + 2, 0:512])
        for i in range(1, 4):
            im[f"xr{i}"] = np.ascontiguousarray(xT[:, :, 512 * i : 512 * i + 512])
        in_maps.append(im)
    return key, in_maps


def _unshuffle_out(o):
    """[128, 16, D] tile-major kernel output -> [S, D]."""
    return np.ascontiguousarray(o.transpose(1, 0, 2)).reshape(S, D)


def _run(x, mask, w_qkv, w_out, trace=False, trace_cores=None):
    key, in_maps = _prepare(x, mask, w_qkv, w_out)
    nc = _get_nc(key)
    res = run_bass_kernel_spmd(
        nc,
        in_maps,
        core_ids=list(range(NCORES)),
        trace=trace,
        trace_cores=trace_cores,
    )
    outs = np.stack(
        [
            sum(
                _unshuffle_out(res.results[4 * b + g]["out"].astype(np.float32))
                for g in range(4)
            )
            for b in range(B)
        ]
    )
    return outs.astype(np.float32), res


def kernel(x, mask, w_qkv, w_out):
    x = np.asarray(x, np.float32)
    mask = np.asarray(mask)
    w_qkv = np.asarray(w_qkv, np.float32)
    w_out = np.asarray(w_out, np.float32)
    out, _ = _run(x, mask, w_qkv, w_out)
    return out
